# revision 32
# baseline (speedup 1.0000x reference)
"""Causal attention head on 8 TRN2 NeuronCores.

reference: out = softmax(causal((x @ wqk) @ x.T)) @ x @ wov
  x: [4096, 1024] f32, wqk/wov: [1024, 1024] f32.

Sharding: sequence-parallel on query rows with stride-8 interleave -- core m
owns global rows {m, m+8, m+16, ...} (512 rows). This balances the causal
triangle perfectly across cores AND keeps the SPMD graph identical on every
core: the causal mask depends on the core only through its input data
(a host-prepared [128, 1024] additive mask), never through the graph.

Per-core layout: 512 local rows = 4 row tiles of 128 (partition dim).
Local row tile r, local row t' -> global row 1024*r + m + 8*t'.
Row tile r attends to columns [0, 1024*(r+1)): col chunks c = 0..2r+1 of 512.
Chunks c = 2r, 2r+1 are the "diagonal" (mask halves 0/1); earlier chunks are
causally full. Each core runs an identical 20-unit S/PV schedule.

Precision: scores must be ~fp32-accurate (softmax of std~1024 logits is
argmax-sensitive; the min top-2 logit gap in this data is 0.1). Q and S
matmuls run in f32r (fp32 data, reduced-precision PE mode): measured on HW
it streams 512-col matmuls in 227ns -- within 5% of fp16 -- and leaves a
logit error of rms ~0.2 (roughly an effective 13-bit mantissa), which the
softmax tolerates (simulated output rel err ~6e-3 vs the 2e-2 gate). This
replaces the previous fp16-main + 2x fp8-DoubleRow-correction scheme, which
cost 2.13x a plain fp16 pass on HW (461ns per k-chunk vs f32r's 227ns).
PV and OV run in plain fp16 (~3e-4 output error floor).

Schedule staggering: the PE work that depends on softmax results (P
transposes, PV) is emitted one xt-chunk later than the data dependency
requires, so VectorE/ScalarE softmax latency hides behind independent S
matmuls and the TensorE never stalls.

Further scheduling measures (each verified against NTFF traces):
- A block of dependency-free identity transposes at kernel start trips
  the PE HAM activity monitor to full clock (2.4GHz) before the DMA-paced Q
  phase; otherwise everything before ~44us runs at the idle 1.2GHz clock.
  Extra heartbeat groups are interleaved into the Q phase and the c-loop
  (c=2..5): during unavoidable DMA waits the filler keeps the activity
  monitor from halving the clock (a bare 3us stall otherwise becomes a
  ~10us one via the clock drop + re-ramp hysteresis).
- Q inputs stream chunked across both the sync and gpsimd DMA rings (a
  whole-tensor DMA would signal completion only at the end and stall the
  first matmul ~22us); xv cache/wov loads are deferred into the c-loop so
  the head bandwidth belongs to Q. The stream front is bandwidth-bound
  (~360GB/s/core): xt c0 is split across both rings, xt c1-c7 ride sync
  (the faster ring), xv cache 8-16 rides sync behind xt c2, and wov +
  spills ride gpsimd. xt tiles are triple-buffered so a chunk's 2MB DMA
  starts ~2 iterations before use.
- Row tile 3 is flash-split: chunks 0-5 get max/exp/transpose/PV as soon as
  c=5 data exists; after the last S chunk only chunks 6-7 plus an
  exp-rescale merge remain, halving the serial tail.
- xv spill tiles beyond the 16-chunk cache are prefetched 1-2 c-iterations
  ahead (3 rotating buffers).
"""
import numpy as np
import ml_dtypes

import concourse.bass as bass
import concourse.tile as tile
from concourse import bacc, mybir
from concourse.bass_utils import run_bass_kernel_spmd
from concourse.masks import make_identity

F32 = mybir.dt.float32
F32R = mybir.dt.float32r
F16 = mybir.dt.float16
BF16 = mybir.dt.bfloat16
E5 = mybir.dt.float8e5

N = 4096          # sequence length
D = 1024          # model dim
CORES = 8
ROWS = N // CORES  # 512 local rows per core
RT = ROWS // 128   # 4 row tiles
KC = D // 128      # 8 contraction chunks
MASK_VAL = -1e9
XV_CACHE = 16      # xv chunks cached for all row tiles
XV_GRP = 4         # xv chunks fetched per DMA beyond the cache

_F16 = np.float16


def build_nc():
    nc = bacc.Bacc("TRN2", target_bir_lowering=False, debug=False,
                   num_devices=CORES)

    def inp(name, shape, dt):
        return nc.dram_tensor(name, shape, dt, kind="ExternalInput").ap()

    # Q inputs ride as fp16: the Q matmul streams at the same rate as f32r
    # (216 vs 227ns per 512 cols) but the head DMA halves (3MB vs 6MB), so
    # the S stream starts ~9us earlier. qt keeps f32r for the S pass; the
    # fp16 input quantization raises logit rms to ~0.34, which the softmax
    # tolerates (measured output rel 1.03e-2 vs the 2e-2 gate).
    wqk_d = inp("wqk_h", [D, D], F16)
    xq_d = inp("xq_h", [D, ROWS], F16)
    xt_d = inp("xt_f", [D, N], F32R)
    xv_d = inp("xv", [N, D], F16)
    wov_d = inp("wov", [D, D], F16)
    mask_d = inp("mask", [128, 1024], BF16)
    out_d = nc.dram_tensor("out", [ROWS, D], F16, kind="ExternalOutput").ap()

    # rearranged views for single-DMA chunked loads: row-block k -> free slot k
    xt_v = xt_d.rearrange("(k p) j -> p k j", p=128)
    xv_v = xv_d.rearrange("(g p) j -> p g j", p=128)
    wov_v = wov_d.rearrange("(k p) j -> p k j", p=128)

    with tile.TileContext(nc) as tc:
        with (
            tc.tile_pool(name="sb", bufs=1) as sb,
            tc.tile_pool(name="sb2", bufs=2) as sb2,
            tc.tile_pool(name="ps_mm", bufs=2, space="PSUM") as ps_mm,
            tc.tile_pool(name="ps_att", bufs=4, space="PSUM") as ps_att,
            tc.tile_pool(name="ps_tp", bufs=2, space="PSUM") as ps_tp,
        ):
            ident = sb.tile([128, 128], F16, tag="ident")
            make_identity(nc, ident[:])
            mask_sb = sb.tile([128, 1024], BF16, tag="mask")

            # dependency-free transposes at kernel start: trips the PE HAM
            # activity monitor to full clock (2.4GHz) before the DMA-paced Q
            # phase starts. The 1-column copy satisfies the BIR verifier's
            # every-write-needs-a-reader rule.
            warmsink = sb.tile([128, 2], F16, tag="warmsink")

            def pe_heartbeat(n, name, pool=None):
                for g in range(n):
                    wp = ((pool or ps_tp)
                          .tile([128, 512], F16,
                                tag="att" if pool is not None else "tp",
                                name=f"warm_{name}_{g}"))
                    for i in range(4):
                        nc.tensor.matmul(wp[:, bass.ts(i, 128)], ident[:],
                                         ident[:], is_transpose=True,
                                         start=(i == 0), stop=(i == 3))
                    nc.vector.tensor_copy(warmsink[:, 0:1], wp[:, 0:1])

            pe_heartbeat(4, "boot", pool=ps_att)

            # ---- Q-side inputs (per-chunk DMAs so PE can start early) ----
            wqk_sb = sb.tile([128, KC * D], F16, tag="wqk_s")
            xq_sb = sb.tile([128, KC * ROWS], F16, tag="xq_xvc")
            # f32 chunks split across BOTH rings: each chunk's completion
            # releases PE work progressively and the two rings deliver in
            # parallel.
            for k in range(KC):
                eng = nc.sync if k < KC // 2 else nc.gpsimd
                eng.dma_start(wqk_sb[:, bass.ts(k, D)],
                              wqk_d[bass.ts(k, 128), :])
                eng.dma_start(xq_sb[:, bass.ts(k, ROWS)],
                              xq_d[bass.ts(k, 128), :])
            # mask is needed only by s_chunk(0,0) at ~35us: load it behind
            # the Q chunks so the head bandwidth is purely Q's
            nc.gpsimd.dma_start(mask_sb[:], mask_d[:])

            # ---- Q phase: QT[d', t] chunks in f32r ----
            qt_f = sb.tile([128, KC * ROWS], F32R, tag="qt_f")

            def q_main(q):
                acc = ps_mm.tile([128, ROWS], F32, tag="mm", name=f"qm{q}")
                for k in range(KC):
                    nc.tensor.matmul(
                        acc[:],
                        wqk_sb[:, k * D + q * 128: k * D + (q + 1) * 128],
                        xq_sb[:, bass.ts(k, ROWS)],
                        start=(k == 0), stop=(k == KC - 1))
                nc.vector.tensor_copy(qt_f[:, bass.ts(q, ROWS)], acc[:])

            for q in range(KC):
                q_main(q)
                # keep the HAM activity window from seeing a fully idle
                # stretch while the Q input stream paces the PE
                pe_heartbeat(1, f"q{q}")

            # ---- persistent per-row-tile buffers ----
            s_off = [0, 1024, 3072, 6144]
            s_len = [(2 * r + 2) * 512 for r in range(RT)]
            s_all = sb.tile([128, 10240], F32, tag="wqk_s")
            xv_cache = sb.tile([128, XV_CACHE * D], F16, tag="xq_xvc")

            mx_all = sb.tile([128, 8 * RT], F32, tag="mx")
            negmax = sb.tile([128, RT], F32, tag="negmax")
            lsum = sb.tile([128, RT], F32, tag="lsum")
            lpart = sb.tile([128, 2 * RT], F32, tag="lpart")
            linv = sb.tile([128, RT], F32, tag="linv")
            # flash-split scratch for row tile 3 (see below):
            # 0 negA, 1 negB, 2 negm, 3 dA, 4 dB, 5 sumA, 6 sumB*dB, 7 l,
            # 8 cA, 9 cB, 10 linv3
            st3 = sb.tile([128, 12], F32, tag="st3")

            wov_sb = sb.tile([128, KC * D], F16, tag="wov")

            xt_cur = {}

            def dma_xt(c):
                # c=0 is needed the moment Q finishes: split it across both
                # rings (each half queues right behind that ring's Q chunks)
                xt32 = sb2.tile([128, KC * 512], F32R, tag="xt32",
                                bufs=3, name=f"xt32_c{c}")
                v = xt32[:].rearrange("p (k j) -> p k j", k=KC)
                if c == 0:
                    h = KC // 2
                    nc.sync.dma_start(v[:, 0:h, :], xt_v[:, 0:h, bass.ts(c, 512)])
                    nc.gpsimd.dma_start(v[:, h:KC, :], xt_v[:, h:KC, bass.ts(c, 512)])
                else:
                    nc.sync.dma_start(v, xt_v[:, :, bass.ts(c, 512)])
                xt_cur["f"] = xt32

            def s_chunk(r, c):
                acc = ps_mm.tile([128, 512], F32, tag="mm",
                                 name=f"s_r{r}c{c}")
                xt32 = xt_cur["f"]
                for k in range(KC):
                    nc.tensor.matmul(
                        acc[:],
                        qt_f[:, k * ROWS + r * 128: k * ROWS + (r + 1) * 128],
                        xt32[:, bass.ts(k, 512)],
                        start=(k == 0), stop=(k == KC - 1))
                dst = s_all[:, s_off[r] + c * 512: s_off[r] + (c + 1) * 512]
                if c == 2 * r:
                    nc.vector.tensor_add(dst, acc[:], mask_sb[:, 0:512])
                elif c == 2 * r + 1:
                    nc.vector.tensor_add(dst, acc[:], mask_sb[:, 512:1024])
                else:
                    nc.scalar.copy(dst, acc[:])
                # per-chunk row max (pipelines the softmax stats)
                nc.vector.tensor_reduce(
                    out=mx_all[:, r * 8 + c: r * 8 + c + 1], in_=dst,
                    op=mybir.AluOpType.max, axis=mybir.AxisListType.X)

            p_tiles = {}
            pt_tiles = {}
            att_tiles = {}
            xv_pending = {}

            def prefetch_xv(bases):
                for base in bases:
                    if base in xv_pending:
                        continue
                    xv_t = sb2.tile([128, XV_GRP * D], F16, tag="xv",
                                    bufs=2, name=f"xv_j{base}")
                    nc.gpsimd.dma_start(
                        xv_t[:].rearrange("p (g j) -> p g j", g=XV_GRP),
                        xv_v[:, base: base + XV_GRP, :])
                    xv_pending[base] = xv_t

            def stats(r):
                """combine chunk maxes -> exp -> row sums (DVE/ACT only)"""
                nm = negmax[:, r: r + 1]
                nc.vector.tensor_reduce(
                    out=nm, in_=mx_all[:, r * 8: r * 8 + 2 * r + 2],
                    op=mybir.AluOpType.max, axis=mybir.AxisListType.X,
                    negate=True)
                # even/odd r alternate two single-buffered regions (their
                # lifetimes interleave exactly) to save SBUF
                p_r = sb2.tile([128, s_len[r]], F16,
                               tag=f"p_{r % 2}", bufs=1, name=f"p_r{r}")
                half = (s_len[r] // 2 + 511) // 512 * 512 if s_len[r] > 2048 else s_len[r]
                pieces = [(0, half)]
                if half < s_len[r]:
                    pieces.append((half, s_len[r] - half))
                for pi, (off, ln) in enumerate(pieces):
                    nc.scalar.activation(
                        p_r[:, off: off + ln],
                        s_all[:, s_off[r] + off: s_off[r] + off + ln],
                        mybir.ActivationFunctionType.Exp,
                        bias=nm, scale=1.0,
                        accum_out=lpart[:, 2 * r + pi: 2 * r + pi + 1])
                if len(pieces) == 1:
                    nc.vector.reciprocal(linv[:, r: r + 1],
                                         lpart[:, 2 * r: 2 * r + 1])
                else:
                    nc.vector.tensor_add(lsum[:, r: r + 1],
                                         lpart[:, 2 * r: 2 * r + 1],
                                         lpart[:, 2 * r + 1: 2 * r + 2])
                    nc.vector.reciprocal(linv[:, r: r + 1], lsum[:, r: r + 1])
                p_tiles[r] = p_r

            def tpv(r):
                """P transposes + PV matmuls (PE-heavy)"""
                p_r = p_tiles[r]
                njc = 8 * (r + 1)
                prefetch_xv(range(XV_CACHE, njc, XV_GRP))
                pt_r = sb2.tile([128, s_len[RT - 1]], F16, tag="pt",
                                name=f"pt_r{r}")
                pt_tiles[r] = pt_r
                nch = s_len[r] // 128
                for g in range(nch // 4):
                    pt_ps = ps_tp.tile([128, 512], F16, tag="tp",
                                       name=f"pt_r{r}g{g}")
                    for i in range(4):
                        jc = g * 4 + i
                        nc.tensor.matmul(
                            pt_ps[:, bass.ts(i, 128)],
                            p_r[:, bass.ts(jc, 128)],
                            ident[:], is_transpose=True,
                            start=(i == 0), stop=(i == 3))
                    nc.vector.tensor_copy(
                        pt_r[:, bass.ts(g, 512)], pt_ps[:])
                att_ps = [ps_att.tile([128, 512], F32, tag="att",
                                      name=f"att_r{r}h{h}")
                          for h in range(2)]
                att_tiles[r] = att_ps
                jc = 0
                while jc < njc:
                    if jc < XV_CACHE:
                        xv_t, base, span = xv_cache, 0, min(XV_CACHE, njc)
                    else:
                        # r=2 peeks (r=3 reuses the same tiles); r<2 never
                        # reaches the spill path
                        xv_t = xv_pending[jc]
                        base, span = jc, XV_GRP
                    for j2 in range(span):
                        lhs = pt_r[:, (base + j2) * 128:
                                   (base + j2 + 1) * 128]
                        for h in range(2):
                            nc.tensor.matmul(
                                att_ps[h][:], lhs,
                                xv_t[:, j2 * D + h * 512: j2 * D + (h + 1) * 512],
                                start=(base + j2 == 0),
                                stop=(base + j2 == njc - 1))
                    jc = base + span

            def fin(r):
                """att normalize + transpose + OV + output DMA"""
                att_ps = att_tiles[r]
                att_sb = sb2.tile([128, D], F16, tag="att_sb", bufs=1,
                                  name=f"att_sb{r}")
                for h in range(2):
                    nc.scalar.mul(att_sb[:, bass.ts(h, 512)], att_ps[h][:],
                                  linv[:, r: r + 1])
                attT = sb2.tile([128, D], F16, tag="att_sb", bufs=1,
                                name=f"attT{r}")
                for g in range(2):
                    at_ps = ps_tp.tile([128, 512], F16, tag="tp",
                                       name=f"at_r{r}g{g}")
                    for i in range(4):
                        nc.tensor.matmul(at_ps[:, bass.ts(i, 128)],
                                         att_sb[:, bass.ts(g * 4 + i, 128)],
                                         ident[:], is_transpose=True,
                                         start=(i == 0), stop=(i == 3))
                    nc.vector.tensor_copy(attT[:, bass.ts(g, 512)], at_ps[:])
                out_sb = sb2.tile([128, D], F16, tag="out_sb", bufs=1,
                                  name=f"out_sb{r}")
                for h in range(2):
                    acc = ps_mm.tile([128, 512], F32, tag="mm",
                                     name=f"ov_r{r}h{h}")
                    for k in range(KC):
                        nc.tensor.matmul(
                            acc[:], attT[:, bass.ts(k, 128)],
                            wov_sb[:, k * D + h * 512: k * D + (h + 1) * 512],
                            start=(k == 0), stop=(k == KC - 1))
                    nc.scalar.copy(out_sb[:, bass.ts(h, 512)], acc[:])
                nc.gpsimd.dma_start(out_d[bass.ts(r, 128), :], out_sb[:])

            # ---- row tile 3 is flash-split: part A (chunks 0-5) gets its
            # own max/exp/PV as soon as c=5 data is ready, so only part B
            # (chunks 6-7) plus a rescale-merge remains after the last S
            # chunk. Cuts the serial tail roughly in half. ----
            def stats_a3():
                nc.vector.tensor_reduce(
                    out=st3[:, 0:1], in_=mx_all[:, 24:30],
                    op=mybir.AluOpType.max, axis=mybir.AxisListType.X,
                    negate=True)
                p_a = sb2.tile([128, s_len[RT - 1]], F16, tag="p_1", bufs=1,
                               name="p_a3")
                for pi in range(2):
                    nc.scalar.activation(
                        p_a[:, pi * 1536:(pi + 1) * 1536],
                        s_all[:, s_off[3] + pi * 1536:
                              s_off[3] + (pi + 1) * 1536],
                        mybir.ActivationFunctionType.Exp,
                        bias=st3[:, 0:1], scale=1.0,
                        accum_out=lpart[:, 6 + pi: 7 + pi])
                p_tiles["a3"] = p_a

            def tpv3_a():
                p_a = p_tiles["a3"]
                pt_r3 = sb2.tile([128, s_len[RT - 1]], F16, tag="pt",
                                 name="pt_r3")
                pt_tiles[3] = pt_r3
                for g in range(6):
                    pt_ps = ps_tp.tile([128, 512], F16, tag="tp",
                                       name=f"pt_a3g{g}")
                    for i in range(4):
                        nc.tensor.matmul(pt_ps[:, bass.ts(i, 128)],
                                         p_a[:, bass.ts(g * 4 + i, 128)],
                                         ident[:], is_transpose=True,
                                         start=(i == 0), stop=(i == 3))
                    nc.vector.tensor_copy(pt_r3[:, bass.ts(g, 512)], pt_ps[:])
                att_a = [ps_att.tile([128, 512], F32, tag="att",
                                     name=f"att_a3h{h}") for h in range(2)]
                att_tiles["a3"] = att_a
                jc = 0
                while jc < 24:
                    if jc < XV_CACHE:
                        xv_t, base, span = xv_cache, 0, XV_CACHE
                    else:
                        xv_t = xv_pending.pop(jc)
                        base, span = jc, XV_GRP
                    for j2 in range(span):
                        lhs = pt_r3[:, (base + j2) * 128:(base + j2 + 1) * 128]
                        for h in range(2):
                            nc.tensor.matmul(
                                att_a[h][:], lhs,
                                xv_t[:, j2 * D + h * 512:
                                     j2 * D + (h + 1) * 512],
                                start=(base + j2 == 0),
                                stop=(base + j2 == 23))
                    jc = base + span

            def stats_b3():
                nc.vector.tensor_reduce(
                    out=st3[:, 1:2], in_=mx_all[:, 30:32],
                    op=mybir.AluOpType.max, axis=mybir.AxisListType.X,
                    negate=True)
                p_b = sb2.tile([128, 1024], F16, tag="p_0", bufs=1,
                               name="p_b3")
                # two 512-col halves: the first transpose group starts while
                # the second half's exp still runs (lpart[0:2] are dead by
                # now -- stats(0) consumed them at c=1)
                for pi in range(2):
                    nc.scalar.activation(
                        p_b[:, pi * 512:(pi + 1) * 512],
                        s_all[:, s_off[3] + 3072 + pi * 512:
                              s_off[3] + 3072 + (pi + 1) * 512],
                        mybir.ActivationFunctionType.Exp,
                        bias=st3[:, 1:2], scale=1.0,
                        accum_out=lpart[:, pi: pi + 1])
                nc.vector.tensor_add(lsum[:, 3:4], lpart[:, 0:1],
                                     lpart[:, 1:2])
                p_tiles["b3"] = p_b
                # merge scalars depend only on the A/B stats -- compute them
                # here so their serial ACT/DVE latency hides under PV-B
                nc.vector.tensor_reduce(out=st3[:, 2:3], in_=st3[:, 0:2],
                                        op=mybir.AluOpType.min,
                                        axis=mybir.AxisListType.X)
                nc.scalar.activation(st3[:, 3:4], st3[:, 0:1],
                                     mybir.ActivationFunctionType.Exp,
                                     bias=st3[:, 2:3], scale=-1.0)
                nc.scalar.activation(st3[:, 4:5], st3[:, 1:2],
                                     mybir.ActivationFunctionType.Exp,
                                     bias=st3[:, 2:3], scale=-1.0)
                nc.vector.tensor_add(st3[:, 5:6], lpart[:, 6:7],
                                     lpart[:, 7:8])
                nc.vector.tensor_scalar_mul(st3[:, 6:7], lsum[:, 3:4],
                                            st3[:, 4:5])
                nc.vector.scalar_tensor_tensor(
                    st3[:, 7:8], st3[:, 5:6], st3[:, 3:4], st3[:, 6:7],
                    mybir.AluOpType.mult, mybir.AluOpType.add)
                nc.vector.reciprocal(st3[:, 10:11], st3[:, 7:8])
                nc.vector.tensor_scalar_mul(st3[:, 8:9], st3[:, 3:4],
                                            st3[:, 10:11])
                nc.vector.tensor_scalar_mul(st3[:, 9:10], st3[:, 4:5],
                                            st3[:, 10:11])

            def tpv3_b():
                p_b = p_tiles["b3"]
                pt_r3 = pt_tiles[3]
                for g in range(2):
                    pt_ps = ps_tp.tile([128, 512], F16, tag="tp",
                                       name=f"pt_b3g{g}")
                    for i in range(4):
                        nc.tensor.matmul(pt_ps[:, bass.ts(i, 128)],
                                         p_b[:, bass.ts(g * 4 + i, 128)],
                                         ident[:], is_transpose=True,
                                         start=(i == 0), stop=(i == 3))
                    nc.vector.tensor_copy(
                        pt_r3[:, 3072 + g * 512: 3072 + (g + 1) * 512],
                        pt_ps[:])
                att_b = [ps_att.tile([128, 512], F32, tag="att",
                                     name=f"att_b3h{h}") for h in range(2)]
                att_tiles["b3"] = att_b
                jc = 24
                while jc < 32:
                    xv_t = xv_pending.pop(jc)
                    for j2 in range(XV_GRP):
                        lhs = pt_r3[:, (jc + j2) * 128:(jc + j2 + 1) * 128]
                        for h in range(2):
                            nc.tensor.matmul(
                                att_b[h][:], lhs,
                                xv_t[:, j2 * D + h * 512:
                                     j2 * D + (h + 1) * 512],
                                start=(jc + j2 == 24),
                                stop=(jc + j2 == 31))
                    jc += XV_GRP

            def fin3():
                """merge A/B partial PVs (scalars precomputed), transpose +
                OV + out"""
                att_a, att_b = att_tiles["a3"], att_tiles["b3"]
                att_sb = sb2.tile([128, D], F16, tag="att_sb", bufs=1,
                                  name="att_sb3")
                for h in range(2):
                    # stage B in SBUF: the fused op may read only one PSUM
                    # input
                    u = sb2.tile([128, 512], F16, tag="u3", bufs=1,
                                 name=f"u3h{h}")
                    nc.vector.tensor_scalar_mul(u[:], att_b[h][:],
                                                st3[:, 9:10])
                    nc.vector.scalar_tensor_tensor(
                        att_sb[:, bass.ts(h, 512)], att_a[h][:], st3[:, 8:9],
                        u[:], mybir.AluOpType.mult, mybir.AluOpType.add)
                attT = sb2.tile([128, D], F16, tag="att_sb", bufs=1,
                                name="attT3")
                for g in range(2):
                    at_ps = ps_tp.tile([128, 512], F16, tag="tp",
                                       name=f"at_r3g{g}")
                    for i in range(4):
                        nc.tensor.matmul(at_ps[:, bass.ts(i, 128)],
                                         att_sb[:, bass.ts(g * 4 + i, 128)],
                                         ident[:], is_transpose=True,
                                         start=(i == 0), stop=(i == 3))
                    nc.vector.tensor_copy(attT[:, bass.ts(g, 512)], at_ps[:])
                out_sb = sb2.tile([128, D], F16, tag="out_sb", bufs=1,
                                  name="out_sb3")
                for h in range(2):
                    acc = ps_mm.tile([128, 512], F32, tag="mm",
                                     name=f"ov_r3h{h}")
                    for k in range(KC):
                        nc.tensor.matmul(
                            acc[:], attT[:, bass.ts(k, 128)],
                            wov_sb[:, k * D + h * 512: k * D + (h + 1) * 512],
                            start=(k == 0), stop=(k == KC - 1))
                    nc.scalar.copy(out_sb[:, bass.ts(h, 512)], acc[:])
                    # per-half DMA: the h=0 transfer drains while the h=1 OV
                    # still runs, shortening the end barrier
                    nc.gpsimd.dma_start(
                        out_d[bass.ts(3, 128), bass.ts(h, 512)],
                        out_sb[:, bass.ts(h, 512)])

            # staggered schedule: S chunks stream; softmax stats right after
            # data ready; PE-dependent tpv/fin one chunk later. Deferred
            # gpsimd-ring loads (xv halves, wov) keep the head clear for the
            # Q-phase inputs; xv spill tiles prefetch 1-2 chunks early.
            HC = XV_CACHE // 4
            for c in range(2 * RT):
                dma_xt(c)
                if c == 0:
                    # cache chunks 0-8 in one DMA (tpv(0) needs the full
                    # range before any of it is useful)
                    nc.gpsimd.dma_start(
                        xv_cache[:, 0:8 * D]
                        .rearrange("p (g j) -> p g j", g=8),
                        xv_v[:, 0:8, :])
                if c == 2:
                    # chunks 8-16 (tpv(1)'s range) + wov ride gpsimd, which
                    # has slack with the fp16-Q head; sync stays clear for
                    # the xt c3/c4 stream
                    nc.gpsimd.dma_start(
                        xv_cache[:, 8 * D:16 * D]
                        .rearrange("p (g j) -> p g j", g=8),
                        xv_v[:, 8:16, :])
                    nc.gpsimd.dma_start(
                        wov_sb[:].rearrange("p (k j) -> p k j", k=KC),
                        wov_v[:])
                if c == 5:
                    prefetch_xv(range(XV_CACHE, 24, XV_GRP))
                if c == 6:
                    prefetch_xv([24])
                if c == 7:
                    prefetch_xv([28])
                # PE work that does NOT need xt chunk c is emitted first:
                # while the S matmuls wait for the chunk DMA, the in-order
                # PE queue drains PV/transpose work instead of stalling (and
                # the HAM activity monitor keeps the clock at full speed)
                if c >= 2 and c % 2 == 0:
                    tpv(c // 2 - 1)
                if c == 6:
                    tpv3_a()
                if c in (2, 3, 4, 5):
                    pe_heartbeat(2, f"c{c}")
                for r in range(c // 2, RT):
                    s_chunk(r, c)
                if c % 2 == 1 and c < 7:
                    stats((c - 1) // 2)
                if c == 5:
                    stats_a3()
                if c == 7:
                    stats_b3()
                if c >= 3 and c % 2 == 1:
                    fin(c // 2 - 1)
            tpv3_b()
            fin3()

    nc.compile()
    return nc


_NC_CACHE = {}


def _get_nc():
    if "nc" not in _NC_CACHE:
        _NC_CACHE["nc"] = build_nc()
    return _NC_CACHE["nc"]


def make_in_maps(x, wqk, wov):
    x = np.ascontiguousarray(x, dtype=np.float32)
    wqk = np.ascontiguousarray(wqk, dtype=np.float32)
    wov = np.ascontiguousarray(wov, dtype=np.float32)

    xt = np.ascontiguousarray(x.T)
    shared = {"xv": np.asarray(x, dtype=_F16),
              "wov": np.asarray(wov, dtype=_F16),
              "wqk_h": np.asarray(wqk, dtype=_F16), "xt_f": xt}

    in_maps = []
    t_idx = np.arange(128)
    c_idx = np.arange(1024)
    for m in range(CORES):
        xq = np.asarray(np.ascontiguousarray(x[m::CORES].T), dtype=_F16)
        mask = np.asarray(
            np.where(c_idx[None, :] <= m + 8 * t_idx[:, None],
                     0.0, MASK_VAL), dtype=ml_dtypes.bfloat16)
        im = dict(shared)
        im.update({"mask": mask, "xq_h": xq})
        in_maps.append(im)
    return in_maps


def _spot_check(x, wqk, wov, out):
    """Cheap CPU verification of a handful of rows (~30ms).

    Guards against a rare transient-garbage hardware execution (observed
    once: NaN output on a first run after device churn). Near-argmax-tie
    rows can legitimately differ by ~0.2, so require only 6 of 8 sampled
    rows to agree -- a genuinely bad run fails on essentially all rows.
    """
    if not np.isfinite(out).all():
        return False
    rows = (7, 517, 1033, 1622, 2050, 2761, 3313, 3999)
    x64 = x.astype(np.float64)
    wqk64 = wqk.astype(np.float64)
    wov64 = wov.astype(np.float64)
    good = 0
    for i in rows:
        q = x64[i] @ wqk64
        s = q @ x64[: i + 1].T
        s -= s.max()
        p = np.exp(s)
        p /= p.sum()
        ref_row = (p @ x64[: i + 1]) @ wov64
        rel = np.linalg.norm(out[i] - ref_row) / (np.linalg.norm(ref_row) + 1e-30)
        if rel < 0.05:
            good += 1
    return good >= 6


def kernel(x, wqk, wov, _trace=False):
    nc = _get_nc()
    in_maps = make_in_maps(x, wqk, wov)
    out = np.empty((N, D), dtype=np.float32)
    for attempt in range(2):
        res = run_bass_kernel_spmd(nc, in_maps, core_ids=list(range(CORES)),
                                   trace=_trace)
        for m in range(CORES):
            out[m::CORES] = res.results[m]["out"].astype(np.float32)
        if _trace:
            kernel.last_results = res
        if _spot_check(x, wqk, wov, out):
            break
    return out


# revision 33
# speedup vs baseline: 1.0075x; 1.0075x over previous
"""Causal attention head on 8 TRN2 NeuronCores.

reference: out = softmax(causal((x @ wqk) @ x.T)) @ x @ wov
  x: [4096, 1024] f32, wqk/wov: [1024, 1024] f32.

Sharding: sequence-parallel on query rows with stride-8 interleave -- core m
owns global rows {m, m+8, m+16, ...} (512 rows). This balances the causal
triangle perfectly across cores AND keeps the SPMD graph identical on every
core: the causal mask depends on the core only through its input data
(a host-prepared [128, 1024] additive mask), never through the graph.

Per-core layout: 512 local rows = 4 row tiles of 128 (partition dim).
Local row tile r, local row t' -> global row 1024*r + m + 8*t'.
Row tile r attends to columns [0, 1024*(r+1)): col chunks c = 0..2r+1 of 512.
Chunks c = 2r, 2r+1 are the "diagonal" (mask halves 0/1); earlier chunks are
causally full. Each core runs an identical 20-unit S/PV schedule.

Precision: scores must be ~fp32-accurate (softmax of std~1024 logits is
argmax-sensitive; the min top-2 logit gap in this data is 0.1). Q and S
matmuls run in f32r (fp32 data, reduced-precision PE mode): measured on HW
it streams 512-col matmuls in 227ns -- within 5% of fp16 -- and leaves a
logit error of rms ~0.2 (roughly an effective 13-bit mantissa), which the
softmax tolerates (simulated output rel err ~6e-3 vs the 2e-2 gate). This
replaces the previous fp16-main + 2x fp8-DoubleRow-correction scheme, which
cost 2.13x a plain fp16 pass on HW (461ns per k-chunk vs f32r's 227ns).
PV and OV run in plain fp16 (~3e-4 output error floor).

Schedule staggering: the PE work that depends on softmax results (P
transposes, PV) is emitted one xt-chunk later than the data dependency
requires, so VectorE/ScalarE softmax latency hides behind independent S
matmuls and the TensorE never stalls.

Further scheduling measures (each verified against NTFF traces):
- A block of dependency-free identity transposes at kernel start trips
  the PE HAM activity monitor to full clock (2.4GHz) before the DMA-paced Q
  phase; otherwise everything before ~44us runs at the idle 1.2GHz clock.
  Extra heartbeat groups are interleaved into the Q phase and the c-loop
  (c=2..5): during unavoidable DMA waits the filler keeps the activity
  monitor from halving the clock (a bare 3us stall otherwise becomes a
  ~10us one via the clock drop + re-ramp hysteresis).
- Q inputs stream chunked across both the sync and gpsimd DMA rings (a
  whole-tensor DMA would signal completion only at the end and stall the
  first matmul ~22us); xv cache/wov loads are deferred into the c-loop so
  the head bandwidth belongs to Q. The stream front is bandwidth-bound
  (~360GB/s/core): xt c0 is split across both rings, xt c1-c7 ride sync
  (the faster ring), xv cache 8-16 rides sync behind xt c2, and wov +
  spills ride gpsimd. xt tiles are triple-buffered so a chunk's 2MB DMA
  starts ~2 iterations before use.
- Row tile 3 is flash-split: chunks 0-5 get max/exp/transpose/PV as soon as
  c=5 data exists; after the last S chunk only chunks 6-7 plus an
  exp-rescale merge remain, halving the serial tail.
- xv spill tiles beyond the 16-chunk cache are prefetched 1-2 c-iterations
  ahead (3 rotating buffers).
"""
import numpy as np
import ml_dtypes

import concourse.bass as bass
import concourse.tile as tile
from concourse import bacc, mybir
from concourse.bass_utils import run_bass_kernel_spmd
from concourse.masks import make_identity

F32 = mybir.dt.float32
F32R = mybir.dt.float32r
F16 = mybir.dt.float16
BF16 = mybir.dt.bfloat16
E5 = mybir.dt.float8e5

N = 4096          # sequence length
D = 1024          # model dim
CORES = 8
ROWS = N // CORES  # 512 local rows per core
RT = ROWS // 128   # 4 row tiles
KC = D // 128      # 8 contraction chunks
MASK_VAL = -1e9
XV_CACHE = 16      # xv chunks cached for all row tiles
XV_GRP = 4         # xv chunks fetched per DMA beyond the cache

_F16 = np.float16


def build_nc():
    nc = bacc.Bacc("TRN2", target_bir_lowering=False, debug=False,
                   num_devices=CORES)

    def inp(name, shape, dt):
        return nc.dram_tensor(name, shape, dt, kind="ExternalInput").ap()

    # Q inputs ride as fp16: the Q matmul streams at the same rate as f32r
    # (216 vs 227ns per 512 cols) but the head DMA halves (3MB vs 6MB), so
    # the S stream starts ~9us earlier. qt keeps f32r for the S pass; the
    # fp16 input quantization raises logit rms to ~0.34, which the softmax
    # tolerates (measured output rel 1.03e-2 vs the 2e-2 gate).
    wqk_d = inp("wqk_h", [D, D], F16)
    xq_d = inp("xq_h", [D, ROWS], F16)
    xt_d = inp("xt_f", [D, N], F32R)
    xv_d = inp("xv", [N, D], F16)
    wov_d = inp("wov", [D, D], F16)
    mask_d = inp("mask", [128, 1024], BF16)
    out_d = nc.dram_tensor("out", [ROWS, D], F16, kind="ExternalOutput").ap()

    # rearranged views for single-DMA chunked loads: row-block k -> free slot k
    xt_v = xt_d.rearrange("(k p) j -> p k j", p=128)
    xv_v = xv_d.rearrange("(g p) j -> p g j", p=128)
    wov_v = wov_d.rearrange("(k p) j -> p k j", p=128)

    with tile.TileContext(nc) as tc:
        with (
            tc.tile_pool(name="sb", bufs=1) as sb,
            tc.tile_pool(name="sb2", bufs=2) as sb2,
            tc.tile_pool(name="ps_mm", bufs=2, space="PSUM") as ps_mm,
            tc.tile_pool(name="ps_att", bufs=4, space="PSUM") as ps_att,
            tc.tile_pool(name="ps_tp", bufs=2, space="PSUM") as ps_tp,
        ):
            ident = sb.tile([128, 128], F16, tag="ident")
            make_identity(nc, ident[:])
            mask_sb = sb.tile([128, 1024], BF16, tag="mask")

            # dependency-free transposes at kernel start: trips the PE HAM
            # activity monitor to full clock (2.4GHz) before the DMA-paced Q
            # phase starts. The 1-column copy satisfies the BIR verifier's
            # every-write-needs-a-reader rule.
            warmsink = sb.tile([128, 2], F16, tag="warmsink")

            def pe_heartbeat(n, name, pool=None):
                for g in range(n):
                    wp = ((pool or ps_tp)
                          .tile([128, 512], F16,
                                tag="att" if pool is not None else "tp",
                                name=f"warm_{name}_{g}"))
                    for i in range(4):
                        nc.tensor.matmul(wp[:, bass.ts(i, 128)], ident[:],
                                         ident[:], is_transpose=True,
                                         start=(i == 0), stop=(i == 3))
                    nc.vector.tensor_copy(warmsink[:, 0:1], wp[:, 0:1])

            pe_heartbeat(7, "boot", pool=ps_att)

            # ---- Q-side inputs (per-chunk DMAs so PE can start early) ----
            wqk_sb = sb.tile([128, KC * D], F16, tag="wqk_s")
            xq_sb = sb.tile([128, KC * ROWS], F16, tag="xq_xvc")
            # f32 chunks split across BOTH rings: each chunk's completion
            # releases PE work progressively and the two rings deliver in
            # parallel.
            for k in range(KC):
                eng = nc.sync if k < KC // 2 else nc.gpsimd
                eng.dma_start(wqk_sb[:, bass.ts(k, D)],
                              wqk_d[bass.ts(k, 128), :])
                eng.dma_start(xq_sb[:, bass.ts(k, ROWS)],
                              xq_d[bass.ts(k, 128), :])
            # mask is needed only by s_chunk(0,0) at ~35us: load it behind
            # the Q chunks so the head bandwidth is purely Q's
            nc.gpsimd.dma_start(mask_sb[:], mask_d[:])

            # ---- Q phase: QT[d', t] chunks in f32r ----
            qt_f = sb.tile([128, KC * ROWS], F32R, tag="qt_f")

            def q_main(q):
                acc = ps_mm.tile([128, ROWS], F32, tag="mm", name=f"qm{q}")
                for k in range(KC):
                    nc.tensor.matmul(
                        acc[:],
                        wqk_sb[:, k * D + q * 128: k * D + (q + 1) * 128],
                        xq_sb[:, bass.ts(k, ROWS)],
                        start=(k == 0), stop=(k == KC - 1))
                nc.vector.tensor_copy(qt_f[:, bass.ts(q, ROWS)], acc[:])

            for q in range(KC):
                q_main(q)
                # keep the HAM activity window from seeing a fully idle
                # stretch while the Q input stream paces the PE
                pe_heartbeat(1, f"q{q}")

            # ---- persistent per-row-tile buffers ----
            s_off = [0, 1024, 3072, 6144]
            s_len = [(2 * r + 2) * 512 for r in range(RT)]
            s_all = sb.tile([128, 10240], F32, tag="wqk_s")
            xv_cache = sb.tile([128, XV_CACHE * D], F16, tag="xq_xvc")

            mx_all = sb.tile([128, 8 * RT], F32, tag="mx")
            negmax = sb.tile([128, RT], F32, tag="negmax")
            lsum = sb.tile([128, RT], F32, tag="lsum")
            lpart = sb.tile([128, 2 * RT], F32, tag="lpart")
            linv = sb.tile([128, RT], F32, tag="linv")
            # flash-split scratch for row tile 3 (see below):
            # 0 negA, 1 negB, 2 negm, 3 dA, 4 dB, 5 sumA, 6 sumB*dB, 7 l,
            # 8 cA, 9 cB, 10 linv3
            st3 = sb.tile([128, 12], F32, tag="st3")

            wov_sb = sb.tile([128, KC * D], F16, tag="wov")

            xt_cur = {}

            def dma_xt(c):
                # c=0 is needed the moment Q finishes: split it across both
                # rings (each half queues right behind that ring's Q chunks)
                xt32 = sb2.tile([128, KC * 512], F32R, tag="xt32",
                                bufs=3, name=f"xt32_c{c}")
                v = xt32[:].rearrange("p (k j) -> p k j", k=KC)
                if c == 0:
                    h = KC // 2
                    nc.sync.dma_start(v[:, 0:h, :], xt_v[:, 0:h, bass.ts(c, 512)])
                    nc.gpsimd.dma_start(v[:, h:KC, :], xt_v[:, h:KC, bass.ts(c, 512)])
                else:
                    nc.sync.dma_start(v, xt_v[:, :, bass.ts(c, 512)])
                xt_cur["f"] = xt32

            def s_chunk(r, c):
                acc = ps_mm.tile([128, 512], F32, tag="mm",
                                 name=f"s_r{r}c{c}")
                xt32 = xt_cur["f"]
                for k in range(KC):
                    nc.tensor.matmul(
                        acc[:],
                        qt_f[:, k * ROWS + r * 128: k * ROWS + (r + 1) * 128],
                        xt32[:, bass.ts(k, 512)],
                        start=(k == 0), stop=(k == KC - 1))
                dst = s_all[:, s_off[r] + c * 512: s_off[r] + (c + 1) * 512]
                if c == 2 * r:
                    nc.vector.tensor_add(dst, acc[:], mask_sb[:, 0:512])
                elif c == 2 * r + 1:
                    nc.vector.tensor_add(dst, acc[:], mask_sb[:, 512:1024])
                else:
                    nc.scalar.copy(dst, acc[:])
                # per-chunk row max (pipelines the softmax stats)
                nc.vector.tensor_reduce(
                    out=mx_all[:, r * 8 + c: r * 8 + c + 1], in_=dst,
                    op=mybir.AluOpType.max, axis=mybir.AxisListType.X)

            p_tiles = {}
            pt_tiles = {}
            att_tiles = {}
            xv_pending = {}

            def prefetch_xv(bases):
                for base in bases:
                    if base in xv_pending:
                        continue
                    xv_t = sb2.tile([128, XV_GRP * D], F16, tag="xv",
                                    bufs=2, name=f"xv_j{base}")
                    nc.gpsimd.dma_start(
                        xv_t[:].rearrange("p (g j) -> p g j", g=XV_GRP),
                        xv_v[:, base: base + XV_GRP, :])
                    xv_pending[base] = xv_t

            def stats(r):
                """combine chunk maxes -> exp -> row sums (DVE/ACT only)"""
                nm = negmax[:, r: r + 1]
                nc.vector.tensor_reduce(
                    out=nm, in_=mx_all[:, r * 8: r * 8 + 2 * r + 2],
                    op=mybir.AluOpType.max, axis=mybir.AxisListType.X,
                    negate=True)
                # even/odd r alternate two single-buffered regions (their
                # lifetimes interleave exactly) to save SBUF
                p_r = sb2.tile([128, s_len[r]], F16,
                               tag=f"p_{r % 2}", bufs=1, name=f"p_r{r}")
                half = (s_len[r] // 2 + 511) // 512 * 512 if s_len[r] > 2048 else s_len[r]
                pieces = [(0, half)]
                if half < s_len[r]:
                    pieces.append((half, s_len[r] - half))
                for pi, (off, ln) in enumerate(pieces):
                    nc.scalar.activation(
                        p_r[:, off: off + ln],
                        s_all[:, s_off[r] + off: s_off[r] + off + ln],
                        mybir.ActivationFunctionType.Exp,
                        bias=nm, scale=1.0,
                        accum_out=lpart[:, 2 * r + pi: 2 * r + pi + 1])
                if len(pieces) == 1:
                    nc.vector.reciprocal(linv[:, r: r + 1],
                                         lpart[:, 2 * r: 2 * r + 1])
                else:
                    nc.vector.tensor_add(lsum[:, r: r + 1],
                                         lpart[:, 2 * r: 2 * r + 1],
                                         lpart[:, 2 * r + 1: 2 * r + 2])
                    nc.vector.reciprocal(linv[:, r: r + 1], lsum[:, r: r + 1])
                p_tiles[r] = p_r

            def tpv(r):
                """P transposes + PV matmuls (PE-heavy)"""
                p_r = p_tiles[r]
                njc = 8 * (r + 1)
                prefetch_xv(range(XV_CACHE, njc, XV_GRP))
                pt_r = sb2.tile([128, s_len[RT - 1]], F16, tag="pt",
                                name=f"pt_r{r}")
                pt_tiles[r] = pt_r
                nch = s_len[r] // 128
                for g in range(nch // 4):
                    pt_ps = ps_tp.tile([128, 512], F16, tag="tp",
                                       name=f"pt_r{r}g{g}")
                    for i in range(4):
                        jc = g * 4 + i
                        nc.tensor.matmul(
                            pt_ps[:, bass.ts(i, 128)],
                            p_r[:, bass.ts(jc, 128)],
                            ident[:], is_transpose=True,
                            start=(i == 0), stop=(i == 3))
                    nc.vector.tensor_copy(
                        pt_r[:, bass.ts(g, 512)], pt_ps[:])
                att_ps = [ps_att.tile([128, 512], F32, tag="att",
                                      name=f"att_r{r}h{h}")
                          for h in range(2)]
                att_tiles[r] = att_ps
                jc = 0
                while jc < njc:
                    if jc < XV_CACHE:
                        xv_t, base, span = xv_cache, 0, min(XV_CACHE, njc)
                    else:
                        # r=2 peeks (r=3 reuses the same tiles); r<2 never
                        # reaches the spill path
                        xv_t = xv_pending[jc]
                        base, span = jc, XV_GRP
                    for j2 in range(span):
                        lhs = pt_r[:, (base + j2) * 128:
                                   (base + j2 + 1) * 128]
                        for h in range(2):
                            nc.tensor.matmul(
                                att_ps[h][:], lhs,
                                xv_t[:, j2 * D + h * 512: j2 * D + (h + 1) * 512],
                                start=(base + j2 == 0),
                                stop=(base + j2 == njc - 1))
                    jc = base + span

            def fin(r):
                """att normalize + transpose + OV + output DMA"""
                att_ps = att_tiles[r]
                att_sb = sb2.tile([128, D], F16, tag="att_sb", bufs=1,
                                  name=f"att_sb{r}")
                for h in range(2):
                    nc.scalar.mul(att_sb[:, bass.ts(h, 512)], att_ps[h][:],
                                  linv[:, r: r + 1])
                attT = sb2.tile([128, D], F16, tag="att_sb", bufs=1,
                                name=f"attT{r}")
                for g in range(2):
                    at_ps = ps_tp.tile([128, 512], F16, tag="tp",
                                       name=f"at_r{r}g{g}")
                    for i in range(4):
                        nc.tensor.matmul(at_ps[:, bass.ts(i, 128)],
                                         att_sb[:, bass.ts(g * 4 + i, 128)],
                                         ident[:], is_transpose=True,
                                         start=(i == 0), stop=(i == 3))
                    nc.vector.tensor_copy(attT[:, bass.ts(g, 512)], at_ps[:])
                out_sb = sb2.tile([128, D], F16, tag="out_sb", bufs=1,
                                  name=f"out_sb{r}")
                for h in range(2):
                    acc = ps_mm.tile([128, 512], F32, tag="mm",
                                     name=f"ov_r{r}h{h}")
                    for k in range(KC):
                        nc.tensor.matmul(
                            acc[:], attT[:, bass.ts(k, 128)],
                            wov_sb[:, k * D + h * 512: k * D + (h + 1) * 512],
                            start=(k == 0), stop=(k == KC - 1))
                    nc.scalar.copy(out_sb[:, bass.ts(h, 512)], acc[:])
                nc.gpsimd.dma_start(out_d[bass.ts(r, 128), :], out_sb[:])

            # ---- row tile 3 is flash-split: part A (chunks 0-5) gets its
            # own max/exp/PV as soon as c=5 data is ready, so only part B
            # (chunks 6-7) plus a rescale-merge remains after the last S
            # chunk. Cuts the serial tail roughly in half. ----
            def stats_a3():
                nc.vector.tensor_reduce(
                    out=st3[:, 0:1], in_=mx_all[:, 24:30],
                    op=mybir.AluOpType.max, axis=mybir.AxisListType.X,
                    negate=True)
                p_a = sb2.tile([128, s_len[RT - 1]], F16, tag="p_1", bufs=1,
                               name="p_a3")
                for pi in range(2):
                    nc.scalar.activation(
                        p_a[:, pi * 1536:(pi + 1) * 1536],
                        s_all[:, s_off[3] + pi * 1536:
                              s_off[3] + (pi + 1) * 1536],
                        mybir.ActivationFunctionType.Exp,
                        bias=st3[:, 0:1], scale=1.0,
                        accum_out=lpart[:, 6 + pi: 7 + pi])
                p_tiles["a3"] = p_a

            def tpv3_a():
                p_a = p_tiles["a3"]
                pt_r3 = sb2.tile([128, s_len[RT - 1]], F16, tag="pt",
                                 name="pt_r3")
                pt_tiles[3] = pt_r3
                for g in range(6):
                    pt_ps = ps_tp.tile([128, 512], F16, tag="tp",
                                       name=f"pt_a3g{g}")
                    for i in range(4):
                        nc.tensor.matmul(pt_ps[:, bass.ts(i, 128)],
                                         p_a[:, bass.ts(g * 4 + i, 128)],
                                         ident[:], is_transpose=True,
                                         start=(i == 0), stop=(i == 3))
                    nc.vector.tensor_copy(pt_r3[:, bass.ts(g, 512)], pt_ps[:])
                att_a = [ps_att.tile([128, 512], F32, tag="att",
                                     name=f"att_a3h{h}") for h in range(2)]
                att_tiles["a3"] = att_a
                jc = 0
                while jc < 24:
                    if jc < XV_CACHE:
                        xv_t, base, span = xv_cache, 0, XV_CACHE
                    else:
                        xv_t = xv_pending.pop(jc)
                        base, span = jc, XV_GRP
                    for j2 in range(span):
                        lhs = pt_r3[:, (base + j2) * 128:(base + j2 + 1) * 128]
                        for h in range(2):
                            nc.tensor.matmul(
                                att_a[h][:], lhs,
                                xv_t[:, j2 * D + h * 512:
                                     j2 * D + (h + 1) * 512],
                                start=(base + j2 == 0),
                                stop=(base + j2 == 23))
                    jc = base + span

            def stats_b3():
                nc.vector.tensor_reduce(
                    out=st3[:, 1:2], in_=mx_all[:, 30:32],
                    op=mybir.AluOpType.max, axis=mybir.AxisListType.X,
                    negate=True)
                p_b = sb2.tile([128, 1024], F16, tag="p_0", bufs=1,
                               name="p_b3")
                # two 512-col halves: the first transpose group starts while
                # the second half's exp still runs (lpart[0:2] are dead by
                # now -- stats(0) consumed them at c=1)
                for pi in range(2):
                    nc.scalar.activation(
                        p_b[:, pi * 512:(pi + 1) * 512],
                        s_all[:, s_off[3] + 3072 + pi * 512:
                              s_off[3] + 3072 + (pi + 1) * 512],
                        mybir.ActivationFunctionType.Exp,
                        bias=st3[:, 1:2], scale=1.0,
                        accum_out=lpart[:, pi: pi + 1])
                nc.vector.tensor_add(lsum[:, 3:4], lpart[:, 0:1],
                                     lpart[:, 1:2])
                p_tiles["b3"] = p_b
                # merge scalars depend only on the A/B stats -- compute them
                # here so their serial ACT/DVE latency hides under PV-B
                nc.vector.tensor_reduce(out=st3[:, 2:3], in_=st3[:, 0:2],
                                        op=mybir.AluOpType.min,
                                        axis=mybir.AxisListType.X)
                nc.scalar.activation(st3[:, 3:4], st3[:, 0:1],
                                     mybir.ActivationFunctionType.Exp,
                                     bias=st3[:, 2:3], scale=-1.0)
                nc.scalar.activation(st3[:, 4:5], st3[:, 1:2],
                                     mybir.ActivationFunctionType.Exp,
                                     bias=st3[:, 2:3], scale=-1.0)
                nc.vector.tensor_add(st3[:, 5:6], lpart[:, 6:7],
                                     lpart[:, 7:8])
                nc.vector.tensor_scalar_mul(st3[:, 6:7], lsum[:, 3:4],
                                            st3[:, 4:5])
                nc.vector.scalar_tensor_tensor(
                    st3[:, 7:8], st3[:, 5:6], st3[:, 3:4], st3[:, 6:7],
                    mybir.AluOpType.mult, mybir.AluOpType.add)
                nc.vector.reciprocal(st3[:, 10:11], st3[:, 7:8])
                nc.vector.tensor_scalar_mul(st3[:, 8:9], st3[:, 3:4],
                                            st3[:, 10:11])
                nc.vector.tensor_scalar_mul(st3[:, 9:10], st3[:, 4:5],
                                            st3[:, 10:11])

            def tpv3_b():
                p_b = p_tiles["b3"]
                pt_r3 = pt_tiles[3]
                for g in range(2):
                    pt_ps = ps_tp.tile([128, 512], F16, tag="tp",
                                       name=f"pt_b3g{g}")
                    for i in range(4):
                        nc.tensor.matmul(pt_ps[:, bass.ts(i, 128)],
                                         p_b[:, bass.ts(g * 4 + i, 128)],
                                         ident[:], is_transpose=True,
                                         start=(i == 0), stop=(i == 3))
                    nc.vector.tensor_copy(
                        pt_r3[:, 3072 + g * 512: 3072 + (g + 1) * 512],
                        pt_ps[:])
                att_b = [ps_att.tile([128, 512], F32, tag="att",
                                     name=f"att_b3h{h}") for h in range(2)]
                att_tiles["b3"] = att_b
                jc = 24
                while jc < 32:
                    xv_t = xv_pending.pop(jc)
                    for j2 in range(XV_GRP):
                        lhs = pt_r3[:, (jc + j2) * 128:(jc + j2 + 1) * 128]
                        for h in range(2):
                            nc.tensor.matmul(
                                att_b[h][:], lhs,
                                xv_t[:, j2 * D + h * 512:
                                     j2 * D + (h + 1) * 512],
                                start=(jc + j2 == 24),
                                stop=(jc + j2 == 31))
                    jc += XV_GRP

            def fin3():
                """merge A/B partial PVs (scalars precomputed), transpose +
                OV + out"""
                att_a, att_b = att_tiles["a3"], att_tiles["b3"]
                att_sb = sb2.tile([128, D], F16, tag="att_sb", bufs=1,
                                  name="att_sb3")
                for h in range(2):
                    # stage B in SBUF: the fused op may read only one PSUM
                    # input
                    u = sb2.tile([128, 512], F16, tag="u3", bufs=1,
                                 name=f"u3h{h}")
                    nc.vector.tensor_scalar_mul(u[:], att_b[h][:],
                                                st3[:, 9:10])
                    nc.vector.scalar_tensor_tensor(
                        att_sb[:, bass.ts(h, 512)], att_a[h][:], st3[:, 8:9],
                        u[:], mybir.AluOpType.mult, mybir.AluOpType.add)
                attT = sb2.tile([128, D], F16, tag="att_sb", bufs=1,
                                name="attT3")
                for g in range(2):
                    at_ps = ps_tp.tile([128, 512], F16, tag="tp",
                                       name=f"at_r3g{g}")
                    for i in range(4):
                        nc.tensor.matmul(at_ps[:, bass.ts(i, 128)],
                                         att_sb[:, bass.ts(g * 4 + i, 128)],
                                         ident[:], is_transpose=True,
                                         start=(i == 0), stop=(i == 3))
                    nc.vector.tensor_copy(attT[:, bass.ts(g, 512)], at_ps[:])
                out_sb = sb2.tile([128, D], F16, tag="out_sb", bufs=1,
                                  name="out_sb3")
                for h in range(2):
                    acc = ps_mm.tile([128, 512], F32, tag="mm",
                                     name=f"ov_r3h{h}")
                    for k in range(KC):
                        nc.tensor.matmul(
                            acc[:], attT[:, bass.ts(k, 128)],
                            wov_sb[:, k * D + h * 512: k * D + (h + 1) * 512],
                            start=(k == 0), stop=(k == KC - 1))
                    nc.scalar.copy(out_sb[:, bass.ts(h, 512)], acc[:])
                    # per-half DMA: the h=0 transfer drains while the h=1 OV
                    # still runs, shortening the end barrier
                    nc.gpsimd.dma_start(
                        out_d[bass.ts(3, 128), bass.ts(h, 512)],
                        out_sb[:, bass.ts(h, 512)])

            # staggered schedule: S chunks stream; softmax stats right after
            # data ready; PE-dependent tpv/fin one chunk later. Deferred
            # gpsimd-ring loads (xv halves, wov) keep the head clear for the
            # Q-phase inputs; xv spill tiles prefetch 1-2 chunks early.
            HC = XV_CACHE // 4
            for c in range(2 * RT):
                dma_xt(c)
                if c == 0:
                    # cache chunks 0-8 in one DMA (tpv(0) needs the full
                    # range before any of it is useful)
                    nc.gpsimd.dma_start(
                        xv_cache[:, 0:8 * D]
                        .rearrange("p (g j) -> p g j", g=8),
                        xv_v[:, 0:8, :])
                if c == 2:
                    # chunks 8-16 (tpv(1)'s range) + wov ride gpsimd, which
                    # has slack with the fp16-Q head; sync stays clear for
                    # the xt c3/c4 stream
                    nc.gpsimd.dma_start(
                        xv_cache[:, 8 * D:16 * D]
                        .rearrange("p (g j) -> p g j", g=8),
                        xv_v[:, 8:16, :])
                    nc.gpsimd.dma_start(
                        wov_sb[:].rearrange("p (k j) -> p k j", k=KC),
                        wov_v[:])
                if c == 5:
                    prefetch_xv(range(XV_CACHE, 24, XV_GRP))
                if c == 6:
                    prefetch_xv([24])
                if c == 7:
                    prefetch_xv([28])
                # PE work that does NOT need xt chunk c is emitted first:
                # while the S matmuls wait for the chunk DMA, the in-order
                # PE queue drains PV/transpose work instead of stalling (and
                # the HAM activity monitor keeps the clock at full speed)
                if c >= 2 and c % 2 == 0:
                    tpv(c // 2 - 1)
                if c == 6:
                    tpv3_a()
                if c in (2, 3, 4, 5):
                    pe_heartbeat(2, f"c{c}")
                for r in range(c // 2, RT):
                    s_chunk(r, c)
                if c % 2 == 1 and c < 7:
                    stats((c - 1) // 2)
                if c == 5:
                    stats_a3()
                if c == 7:
                    stats_b3()
                if c >= 3 and c % 2 == 1:
                    fin(c // 2 - 1)
            tpv3_b()
            fin3()

    nc.compile()
    return nc


_NC_CACHE = {}


def _get_nc():
    if "nc" not in _NC_CACHE:
        _NC_CACHE["nc"] = build_nc()
    return _NC_CACHE["nc"]


def make_in_maps(x, wqk, wov):
    x = np.ascontiguousarray(x, dtype=np.float32)
    wqk = np.ascontiguousarray(wqk, dtype=np.float32)
    wov = np.ascontiguousarray(wov, dtype=np.float32)

    xt = np.ascontiguousarray(x.T)
    shared = {"xv": np.asarray(x, dtype=_F16),
              "wov": np.asarray(wov, dtype=_F16),
              "wqk_h": np.asarray(wqk, dtype=_F16), "xt_f": xt}

    in_maps = []
    t_idx = np.arange(128)
    c_idx = np.arange(1024)
    for m in range(CORES):
        xq = np.asarray(np.ascontiguousarray(x[m::CORES].T), dtype=_F16)
        mask = np.asarray(
            np.where(c_idx[None, :] <= m + 8 * t_idx[:, None],
                     0.0, MASK_VAL), dtype=ml_dtypes.bfloat16)
        im = dict(shared)
        im.update({"mask": mask, "xq_h": xq})
        in_maps.append(im)
    return in_maps


def _spot_check(x, wqk, wov, out):
    """Cheap CPU verification of a handful of rows (~30ms).

    Guards against a rare transient-garbage hardware execution (observed
    once: NaN output on a first run after device churn). Near-argmax-tie
    rows can legitimately differ by ~0.2, so require only 6 of 8 sampled
    rows to agree -- a genuinely bad run fails on essentially all rows.
    """
    if not np.isfinite(out).all():
        return False
    rows = (7, 517, 1033, 1622, 2050, 2761, 3313, 3999)
    x64 = x.astype(np.float64)
    wqk64 = wqk.astype(np.float64)
    wov64 = wov.astype(np.float64)
    good = 0
    for i in rows:
        q = x64[i] @ wqk64
        s = q @ x64[: i + 1].T
        s -= s.max()
        p = np.exp(s)
        p /= p.sum()
        ref_row = (p @ x64[: i + 1]) @ wov64
        rel = np.linalg.norm(out[i] - ref_row) / (np.linalg.norm(ref_row) + 1e-30)
        if rel < 0.05:
            good += 1
    return good >= 6


def kernel(x, wqk, wov, _trace=False):
    nc = _get_nc()
    in_maps = make_in_maps(x, wqk, wov)
    out = np.empty((N, D), dtype=np.float32)
    for attempt in range(2):
        res = run_bass_kernel_spmd(nc, in_maps, core_ids=list(range(CORES)),
                                   trace=_trace)
        for m in range(CORES):
            out[m::CORES] = res.results[m]["out"].astype(np.float32)
        if _trace:
            kernel.last_results = res
        if _spot_check(x, wqk, wov, out):
            break
    return out
